# revision 1
# baseline (speedup 1.0000x reference)
"""Trainium2 Bass kernel for nn_CounterfactualReasoner (GNN message passing).

Strategy (per sharding hint, node-sharded variant):
 - Host: sort edges by dst ("shard by node" — each core owns a contiguous
   dst range, so all edges of one destination land on one core). Pad each
   core to 100352 edges.
 - Device, per core: indirect-gather z[src]/z[dst] rows, PE-transpose to
   feature-on-partition layout, run the fused MLP (W2@W3a prefolded on host),
   sigmoid*|lr|.  Segment max over dst becomes prefix/suffix max-doubling
   over sorted runs (halo columns carry runs across partition chunks) —
   no collective needed since dst runs never span cores.
 - Host: un-permute the per-core outputs back to original edge order.
"""
import sys
import numpy as np

sys.path.insert(0, "/opt/trn_rl_repo")

# ---------------- configs ----------------
class _Cfg:
    def __init__(self, NPAD, C, CHUNKS, HALO):
        self.NPAD = NPAD          # z rows (padded, mult of 128)
        self.C = C                # edge columns per partition (EC = 128*C)
        self.CHUNKS = CHUNKS      # gather chunks (2048 edges each)
        self.HALO = HALO          # halo width >= max in-degree
        self.F32R = False
        self.DEBUG = False
        self.GS = 1  # slots per indirect gather instruction
        self.SLOTS = 16           # gather rows per partition per chunk
        self.TPC = 4              # 512-edge tiles per chunk
        self.TILE = 512
        self.EC = 128 * C
        self.W = C + 2 * HALO
        ks, k = [], 1
        while k < HALO:
            ks.append(k); k *= 2
        ks.append(HALO // 1 if k == HALO else k)  # ensure coverage of HALO-1
        # steps 1,2,4,...,HALO/2 give lookahead HALO-1
        self.KSTEPS = []
        k = 1
        while k <= HALO // 2:
            self.KSTEPS.append(k); k *= 2
        assert C == CHUNKS * self.SLOTS
        assert HALO & (HALO - 1) == 0

FULL = _Cfg(NPAD=50176, C=784, CHUNKS=49, HALO=64)
NCORES = 8
H = 128
N_NODES = 50000
E_REAL = 800000
PADDST = 4194304.0

_prog_cache = {}


def build_program(P):
    import concourse.bass as bass
    import concourse.bacc as bacc
    import concourse.mybir as mybir
    import concourse.tile as tile
    from contextlib import ExitStack

    f32 = mybir.dt.float32
    f32r = mybir.dt.float32r
    i32 = mybir.dt.int32
    AF = mybir.ActivationFunctionType
    OP = mybir.AluOpType

    def rc(ap):
        return ap.bitcast(f32r) if P.F32R else ap

    nc = bacc.Bacc("TRN2", target_bir_lowering=False, debug=False,
                   enable_asserts=False, num_devices=NCORES)

    z = nc.dram_tensor("z", [P.NPAD, H], f32, kind="ExternalInput")
    srcidx = nc.dram_tensor("srcidx", [P.C, 128], i32, kind="ExternalInput")
    dstidx = nc.dram_tensor("dstidx", [P.C, 128], i32, kind="ExternalInput")
    lrmm = nc.dram_tensor("lrmm", [P.CHUNKS, P.TPC, P.TILE], f32, kind="ExternalInput")
    dstx = nc.dram_tensor("dstx", [128, P.W], f32, kind="ExternalInput")
    alr = nc.dram_tensor("alr", [128, P.C], f32, kind="ExternalInput")
    w1a = nc.dram_tensor("w1a", [H, H], f32, kind="ExternalInput")
    w1r = nc.dram_tensor("w1r", [1, H], f32, kind="ExternalInput")
    w23 = nc.dram_tensor("w23", [H, H], f32, kind="ExternalInput")
    w3b = nc.dram_tensor("w3b", [H, H], f32, kind="ExternalInput")
    w4 = nc.dram_tensor("w4", [H, 1], f32, kind="ExternalInput")
    b1 = nc.dram_tensor("b1", [H, 1], f32, kind="ExternalInput")
    b3p = nc.dram_tensor("b3p", [H, 1], f32, kind="ExternalInput")
    b4b = nc.dram_tensor("b4b", [H, 1], f32, kind="ExternalInput")
    ident = nc.dram_tensor("ident", [128, 128], f32, kind="ExternalInput")
    outb = nc.dram_tensor("outb", [128, P.C], f32, kind="ExternalOutput")
    if P.DEBUG:
        dbg_e4 = nc.dram_tensor("dbg_e4", [128, P.C], f32, kind="ExternalOutput")
        dbg_v = nc.dram_tensor("dbg_v", [128, P.W], f32, kind="ExternalOutput")
        dbg_m = nc.dram_tensor("dbg_m", [128, P.C], f32, kind="ExternalOutput")
        dbg_zst = nc.dram_tensor("dbg_zst", [128, 512], f32, kind="ExternalOutput")

    with tile.TileContext(nc) as tc, ExitStack() as ctx:
        const = ctx.enter_context(tc.tile_pool(name="const", bufs=1))
        stage = ctx.enter_context(tc.tile_pool(name="stage", bufs=2))
        acts = ctx.enter_context(tc.tile_pool(name="acts", bufs=3))
        segp = ctx.enter_context(tc.tile_pool(name="segp", bufs=2))
        pT = ctx.enter_context(tc.tile_pool(name="pT", bufs=3, space="PSUM"))
        p1 = ctx.enter_context(tc.tile_pool(name="p1", bufs=1, space="PSUM"))
        p3 = ctx.enter_context(tc.tile_pool(name="p3", bufs=2, space="PSUM"))
        p4 = ctx.enter_context(tc.tile_pool(name="p4", bufs=1, space="PSUM"))
        gp = ctx.enter_context(tc.tile_pool(name="gp", bufs=3 * 16))

        def load(dram, shape, dtype=f32, nm="t"):
            t = const.tile(shape, dtype, tag=nm, name=nm)
            nc.sync.dma_start(t[:], dram.ap())
            return t

        w1a_s = load(w1a, [H, H], nm="w1a_s")
        w1r_s = load(w1r, [1, H], nm="w1r_s")
        w23_s = load(w23, [H, H], nm="w23_s")
        w3b_s = load(w3b, [H, H], nm="w3b_s")
        w4_s = load(w4, [H, 1], nm="w4_s")
        b1_s = load(b1, [H, 1], nm="b1_s")
        b3p_s = load(b3p, [H, 1], nm="b3p_s")
        b4b_s = load(b4b, [H, 1], nm="b4b_s")
        ident_s = load(ident, [128, 128], nm="ident_s")
        dstx_s = load(dstx, [128, P.W], nm="dstx_s")
        alr_s = load(alr, [128, P.C], nm="alr_s")

        psum4a = p4.tile([128, 512], f32, tag="p4a", name="psum4a")
        psum4b = p4.tile([128, 512], f32, tag="p4b", name="psum4b")
        E4 = const.tile([128, P.C], f32, tag="E4", name="E4")

        S = P.SLOTS
        for c in range(P.CHUNKS):
            zs_sl, zd_sl = [], []
            for k in range(S):
                j = c * S + k
                ixs = gp.tile([128, 1], i32, tag="ixs", name=f"ixs{j}")
                nc.sync.dma_start(ixs[:], srcidx.ap()[j, :, None])
                zsk = gp.tile([128, H], f32, tag="zs", name=f"zs{j}")
                nc.gpsimd.indirect_dma_start(
                    out=zsk[:], out_offset=None, in_=z.ap(),
                    in_offset=bass.IndirectOffsetOnAxis(ap=ixs[:, :1], axis=0))
                zs_sl.append(zsk)
                ixd = gp.tile([128, 1], i32, tag="ixd", name=f"ixd{j}")
                nc.sync.dma_start(ixd[:], dstidx.ap()[j, :, None])
                zdk = gp.tile([128, H], f32, tag="zd", name=f"zd{j}")
                nc.gpsimd.indirect_dma_start(
                    out=zdk[:], out_offset=None, in_=z.ap(),
                    in_offset=bass.IndirectOffsetOnAxis(ap=ixd[:, :1], axis=0))
                zd_sl.append(zdk)
            for u in range(P.TPC):
                t_glob = c * P.TPC + u
                lr_s = stage.tile([1, P.TILE], f32, tag="lr", name=f"lr{t_glob}")
                nc.sync.dma_start(lr_s[:], lrmm.ap()[c, u:u + 1, :])
                pts = pT.tile([128, 512], f32, tag="pT", name=f"pts{t_glob}")
                for s in range(4):
                    nc.tensor.matmul(
                        out=pts[:, s * 128:(s + 1) * 128],
                        lhsT=zs_sl[u * 4 + s][:],
                        rhs=ident_s[:], is_transpose=True)
                zsT = acts.tile([128, 512], f32, tag="zsT", name=f"zsT{t_glob}")
                nc.vector.tensor_copy(zsT[:], pts[:])
                if P.DEBUG and t_glob == 0:
                    nc.sync.dma_start(dbg_zst.ap(), zsT[:])
                ptd = pT.tile([128, 512], f32, tag="pT", name=f"ptd{t_glob}")
                for s in range(4):
                    nc.tensor.matmul(
                        out=ptd[:, s * 128:(s + 1) * 128],
                        lhsT=zd_sl[u * 4 + s][:],
                        rhs=ident_s[:], is_transpose=True)
                zdT = acts.tile([128, 512], f32, tag="zdT", name=f"zdT{t_glob}")
                nc.vector.tensor_copy(zdT[:], ptd[:])

                ps1 = p1.tile([128, 512], f32, tag="p1", name=f"ps1_{t_glob}")
                nc.tensor.matmul(out=ps1[:], lhsT=rc(w1a_s[:]),
                                 rhs=rc(zsT[:]), start=True, stop=False)
                nc.tensor.matmul(out=ps1[:], lhsT=rc(w1r_s[:]),
                                 rhs=rc(lr_s[:]),
                                 start=False, stop=True)
                g1 = acts.tile([128, 512], f32, tag="g1", name=f"g1_{t_glob}")
                nc.scalar.activation(g1[:], ps1[:], AF.Gelu, bias=b1_s[:])

                ps3 = p3.tile([128, 512], f32, tag="p3", name=f"ps3_{t_glob}")
                nc.tensor.matmul(out=ps3[:], lhsT=rc(w23_s[:]),
                                 rhs=rc(g1[:]), start=True, stop=False)
                nc.tensor.matmul(out=ps3[:], lhsT=rc(w3b_s[:]),
                                 rhs=rc(zdT[:]), start=False, stop=True)
                g3 = acts.tile([128, 512], f32, tag="g3", name=f"g3_{t_glob}")
                nc.scalar.activation(g3[:], ps3[:], AF.Gelu, bias=b3p_s[:])

                for s in range(4):
                    q = t_glob * 4 + s
                    tgt = psum4a if q < 512 else psum4b
                    col = q % 512
                    nc.tensor.matmul(out=tgt[:, col:col + 1],
                                     lhsT=rc(g3[:, s * 128:(s + 1) * 128]),
                                     rhs=rc(w4_s[:]), start=True, stop=True)

        nA = min(P.C, 512)
        nc.vector.tensor_copy(E4[:, 0:nA], psum4a[:, 0:nA])
        if P.C > 512:
            nc.vector.tensor_copy(E4[:, 512:P.C], psum4b[:, 0:P.C - 512])

        HA, C, W = P.HALO, P.C, P.W
        Vx = const.tile([128, W], f32, tag="Vx", name="Vx")
        nc.scalar.activation(Vx[:, HA:HA + C], E4[:], AF.Sigmoid, bias=b4b_s[:])
        nc.vector.tensor_tensor(out=Vx[:, HA:HA + C], in0=Vx[:, HA:HA + C],
                                in1=alr_s[:], op=OP.mult)
        # halos: left = prev partition's last HA main cols; right = next's first
        nc.vector.memset(Vx[:, 0:HA], 0.0)
        nc.sync.dma_start(Vx[1:128, 0:HA], Vx[0:127, C:C + HA])
        nc.vector.memset(Vx[:, HA + C:W], 0.0)
        nc.sync.dma_start(Vx[0:127, HA + C:W], Vx[1:128, HA:2 * HA])

        Scur = Vx
        Pcur = Vx
        for k in P.KSTEPS:
            eq = segp.tile([128, W], f32, tag="eq", name=f"eq{k}")
            nc.vector.tensor_tensor(out=eq[:, 0:W - k], in0=dstx_s[:, 0:W - k],
                                    in1=dstx_s[:, k:W], op=OP.is_equal)
            tmp = segp.tile([128, W], f32, tag="tmpS", name=f"tmS{k}")
            nc.vector.tensor_tensor(out=tmp[:, 0:W - k], in0=Scur[:, k:W],
                                    in1=eq[:, 0:W - k], op=OP.mult)
            Snew = segp.tile([128, W], f32, tag="S", name=f"S{k}")
            nc.vector.tensor_tensor(out=Snew[:, 0:W - k], in0=Scur[:, 0:W - k],
                                    in1=tmp[:, 0:W - k], op=OP.max)
            nc.vector.tensor_copy(Snew[:, W - k:W], Scur[:, W - k:W])
            tmp2 = segp.tile([128, W], f32, tag="tmpP", name=f"tmP{k}")
            nc.vector.tensor_tensor(out=tmp2[:, 0:W - k], in0=Pcur[:, 0:W - k],
                                    in1=eq[:, 0:W - k], op=OP.mult)
            Pnew = segp.tile([128, W], f32, tag="P", name=f"P{k}")
            nc.vector.tensor_tensor(out=Pnew[:, k:W], in0=Pcur[:, k:W],
                                    in1=tmp2[:, 0:W - k], op=OP.max)
            nc.vector.tensor_copy(Pnew[:, 0:k], Pcur[:, 0:k])
            Scur, Pcur = Snew, Pnew

        if P.DEBUG:
            nc.sync.dma_start(dbg_e4.ap(), E4[:])
            nc.sync.dma_start(dbg_v.ap(), Vx[:])
        M = segp.tile([128, C], f32, tag="M", name="M")
        nc.vector.tensor_tensor(out=M[:], in0=Scur[:, HA:HA + C],
                                in1=Pcur[:, HA:HA + C], op=OP.max)
        if P.DEBUG:
            nc.sync.dma_start(dbg_m.ap(), M[:])
        nc.vector.tensor_scalar_max(M[:], M[:], 1e-37)
        R = segp.tile([128, C], f32, tag="R", name="R")
        nc.vector.reciprocal(R[:], M[:])
        OUTT = segp.tile([128, C], f32, tag="OUTT", name="OUTT")
        nc.vector.tensor_tensor(out=OUTT[:], in0=Vx[:, HA:HA + C], in1=R[:],
                                op=OP.mult)
        nc.sync.dma_start(outb.ap(), OUTT[:])

    nc.compile()
    return nc


def host_prep(P, z, edge_index, lr_scores, W1, b1, W2, b2, W3, b3, W4, b4):
    """Shard/sort/pad inputs; returns (in_maps, reassembly info)."""
    f32 = np.float32
    src = np.asarray(edge_index[0])
    dst = np.asarray(edge_index[1])
    lr = np.asarray(lr_scores, dtype=f32)
    E = src.shape[0]
    n_nodes = np.asarray(z).shape[0]

    order = np.argsort(dst, kind="stable")
    dst_sorted = dst[order]
    # core boundaries aligned to run boundaries
    pos = [0]
    for c in range(1, NCORES):
        b = c * E // NCORES
        while b < E and dst_sorted[b] == dst_sorted[b - 1]:
            b += 1
        pos.append(b)
    pos.append(E)

    zp = np.zeros((P.NPAD, H), f32)
    zp[:n_nodes] = np.asarray(z, dtype=f32)

    W1 = np.asarray(W1, f32); W2 = np.asarray(W2, f32); W3 = np.asarray(W3, f32)
    W4 = np.asarray(W4, f32)
    b1 = np.asarray(b1, f32); b2 = np.asarray(b2, f32); b3 = np.asarray(b3, f32)
    b4 = np.asarray(b4, f32)
    wmap = {
        "w1a": np.ascontiguousarray(W1[:H]),
        "w1r": np.ascontiguousarray(W1[H:H + 1]),
        "w23": np.ascontiguousarray(W2 @ W3[:H]),
        "w3b": np.ascontiguousarray(W3[H:]),
        "w4": np.ascontiguousarray(W4),
        "b1": b1.reshape(H, 1).copy(),
        "b3p": (b2 @ W3[:H] + b3).reshape(H, 1).astype(f32),
        "b4b": np.full((H, 1), b4[0], f32),
        "ident": np.eye(128, dtype=f32),
        "z": zp,
    }

    in_maps = []
    infos = []
    C, HA, W = P.C, P.HALO, P.W
    for c in range(NCORES):
        idx_c = order[pos[c]:pos[c + 1]]
        n_c = len(idx_c)
        assert n_c <= P.EC, f"core {c} has {n_c} edges > {P.EC}"
        srcS = np.zeros(P.EC, np.int64); srcS[:n_c] = src[idx_c]
        dstS = np.zeros(P.EC, np.int64); dstS[:n_c] = dst[idx_c]
        lrS = np.zeros(P.EC, f32); lrS[:n_c] = lr[idx_c]
        dstC = np.full(P.EC, PADDST, f32); dstC[:n_c] = dst[idx_c].astype(f32)
        if n_c > 1:
            runlens = np.diff(np.flatnonzero(
                np.concatenate(([True], dst[idx_c][1:] != dst[idx_c][:-1], [True]))))
            assert runlens.max() <= HA, f"run {runlens.max()} > halo {HA}"

        dmain = dstC.reshape(128, C)
        dstx_np = np.empty((128, W), f32)
        dstx_np[:, HA:HA + C] = dmain
        dstx_np[1:, :HA] = dmain[:-1, C - HA:]
        dstx_np[0, :HA] = -7.0
        dstx_np[:-1, HA + C:] = dmain[1:, :HA]
        dstx_np[127, HA + C:] = -8.0

        lr_dev = lrS.reshape(128, C).T.ravel()
        m = dict(wmap)
        m["srcidx"] = np.ascontiguousarray(srcS.reshape(128, C).T.astype(np.int32))
        m["dstidx"] = np.ascontiguousarray(dstS.reshape(128, C).T.astype(np.int32))
        m["lrmm"] = np.ascontiguousarray(lr_dev.reshape(P.CHUNKS, P.TPC, P.TILE))
        m["dstx"] = dstx_np
        m["alr"] = np.abs(lrS).reshape(128, C).copy()
        in_maps.append(m)
        infos.append((idx_c, n_c))
    return in_maps, infos, E


def kernel(**inputs) -> np.ndarray:
    P = FULL
    in_maps, infos, E = host_prep(P, **inputs)
    if "full" not in _prog_cache:
        _prog_cache["full"] = build_program(P)
    nc = _prog_cache["full"]
    from concourse import bass_utils
    res = bass_utils.run_bass_kernel_spmd(
        nc, in_maps, core_ids=list(range(NCORES)), trace=False)
    out = np.empty(E, np.float32)
    for c in range(NCORES):
        idx_c, n_c = infos[c]
        out[idx_c] = res.results[c]["outb"].ravel()[:n_c]
    return out

